# revision 24
# baseline (speedup 1.0000x reference)
"""FP8StaticLinear Trainium2 kernel.

out = requant_fp8(qdq_fp8(x, s_in) @ (w * s_w).T + bias, s_out)

Sharding: data-parallel over tokens (B*S=16384 -> 2048/core on 8 cores).
Device math: fp8e4 DoubleRow matmuls on the PE array. Both operands are
halved on entry so the OCP-e4m3fn grid (max 448) maps onto TRN fp8e4
(max 240) exactly; scales are folded back in the epilogue.

x is quantized to fp8 on the host (exact same RNE rounding as the
device DVE path) and uploaded pre-tiled, which cuts the activation DMA
4x and removes the on-device quantize pipeline entirely; the warm-up
phase is then PE-bound instead of DMA-starved.
"""

import numpy as np
import ml_dtypes

import concourse.bass as bass
import concourse.mybir as mybir
from concourse.tile import TileContext
from concourse.vector_clock import ScopedClock
from concourse.bass_utils import run_bass_kernel_spmd

FP8 = mybir.dt.float8e4
F32 = mybir.dt.float32
NP_FP8 = ml_dtypes.float8_e4m3  # TRN fp8e4 (max 240, has inf)

N_CORES = 8
P = 128


# ---------------------------------------------------------------------------
# Workaround: this walrus build rejects >1 sem-wait on the Tile tail Drain
# ("Too many sync wait commands"). Split the waits across single-wait drains.
def _drain_and_barrier(self, tick_clock, wait_clock):
    drain_inst = self.nc.sync.drain()
    wait_clock.add_sem_waits(
        drain_inst.ins, ScopedClock({None: tick_clock.global_clock})
    )
    w = list(drain_inst.ins.sync_info.on_wait)
    if len(w) > 1:
        drain_inst.ins.sync_info = mybir.SyncInfo(on_wait=[w[0]], on_update=[])
        for extra in w[1:]:
            d2 = self.nc.sync.drain()
            d2.ins.sync_info = mybir.SyncInfo(on_wait=[extra], on_update=[])
    self.nc.all_engine_barrier()
    assert self.sems is not None
    popped = self.nc._tile_sem_poison_stack.pop()
    assert popped is self._sem_poison
    self.nc.clear_and_free_semaphores(list(self.sems.allocated().values()))
    self.nc.all_engine_barrier()


TileContext._drain_and_barrier = _drain_and_barrier


def split_sync_waits(nc, max_waits=1):
    """Hoist extra sem-waits onto standalone EventSemaphore carriers."""
    n_new = 0
    for f in nc.m.functions:
        for blk in f.blocks:
            out = []
            changed = False
            for inst in blk.instructions:
                si = inst.sync_info
                w = list(si.on_wait) if si is not None else []
                if len(w) > max_waits:
                    upd = list(inst.sync_info.on_update)
                    for wi in w[max_waits:]:
                        es = mybir.InstEventSemaphore(
                            name=f"hoistw-{n_new}", ins=[], outs=[]
                        )
                        n_new += 1
                        es.engine = inst.engine
                        es.sync_info = mybir.SyncInfo(on_wait=[wi], on_update=[])
                        out.append(es)
                    inst.sync_info = mybir.SyncInfo(
                        on_wait=w[:max_waits], on_update=upd
                    )
                    changed = True
                out.append(inst)
            if changed:
                blk.instructions = out
    return nc
# ---------------------------------------------------------------------------

DR = mybir.MatmulPerfMode.DoubleRow


def build(K, M, N, MF=512):
    """One-core program: out_t[N, M] = requantized (x @ w.T + b) transposed.

    DRAM inputs:
      qxt    [MB, P, KS, MF] fp8   halved-quantized x, block-tiled:
                     qxt[mb, p, j, m] = fp8(clip(x[mb*MF+m, j*128+p]/(2si)))
      wt     [NT, P, KS, P]  fp8   halved weight, tiled:
                     wt[nt, p, j, n] = fp8(w[nt*128+n, j*128+p] / 2)
      bias2  [N]     f32    bias / (2*s_out)
      alpha, two_os  [1, 1] f32:   2*s_in*s_w/s_out,  2*s_out
    Output:
      out_t  [N, M]  f32
    """
    KS = K // P          # 32 k subtiles of 128
    JP = KS // 2         # 16 DoubleRow pairs
    NT = N // P          # 32 n tiles
    MB = M // MF         # 4 m blocks
    NW = min(8, NT)      # warm groups (one psum bank each)
    QW = KS // 4         # k-subtiles per quarter piece
    AF = mybir.ActivationFunctionType
    OP = mybir.AluOpType

    nc = bass.Bass()
    qxt = nc.dram_tensor("qxt", [MB, P, KS, MF], FP8, kind="ExternalInput")
    wt = nc.dram_tensor("wt", [NT, P, KS, P], FP8, kind="ExternalInput")
    # packed per-partition constants: [:, 0:NT] = bias/(2os) tiled,
    # [:, NT] = alpha, [:, NT+1] = 2os
    cst_d = nc.dram_tensor("cst", [P, NT + 2], F32, kind="ExternalInput")
    out_t = nc.dram_tensor("out_t", [N, M], F32, kind="ExternalOutput")

    with TileContext(nc) as tc:
        with (
            tc.tile_pool(name="consts", bufs=1) as consts,
            tc.tile_pool(name="wres", bufs=1) as wres,
            tc.tile_pool(name="qx", bufs=2) as qxp,
            tc.tile_pool(name="psum", bufs=8, space="PSUM") as psp,
            tc.tile_pool(name="epi", bufs=3) as epi,
            tc.tile_pool(name="q8", bufs=3) as q8p,
            tc.tile_pool(name="yout", bufs=4) as yp,
        ):
            # ---- consts: one contiguous DMA on the gpsimd SWDGE queue ----
            cst = consts.tile([P, NT + 2], F32)
            nc.gpsimd.dma_start(cst[:], cst_d[:, :])

            w_tiles = [
                wres.tile([P, KS, P], FP8, tag=f"w{nt}", name=f"w{nt}")
                for nt in range(NT)
            ]
            qx_tiles = {0: qxp.tile([P, KS, MF], FP8, tag="qx", name="qx0")}

            # Loads: qx on the sync HW queue, warm weights on the scalar
            # HW queue (few issues, done long before the first epilogue
            # ACTIVATE needs the engine — a long issue backlog there
            # blocks PSUM recycling). First pieces are small so the first
            # matmul fires as early as possible.
            HW = KS // 2
            for h in range(2):
                nc.sync.dma_start(
                    qx_tiles[0][:, 2 * h * QW : (2 * h + 1) * QW, :],
                    qxt[0, :, 2 * h * QW : (2 * h + 1) * QW, :],
                )
                nc.sync.dma_start(
                    qx_tiles[0][:, (2 * h + 1) * QW : (2 * h + 2) * QW, :],
                    qxt[0, :, (2 * h + 1) * QW : (2 * h + 2) * QW, :],
                )
                for g in range(NW):
                    nc.sync.dma_start(
                        w_tiles[g][:, h * HW : (h + 1) * HW, :],
                        wt[g, :, h * HW : (h + 1) * HW, :],
                    )
            # Remaining weight tiles: one big descriptor each (4KB lines).
            for nt in range(NW, NT):
                nc.sync.dma_start(w_tiles[nt][:], wt[nt, :, :, :])

            def emit_mms(ps, nt, qx, m0=0, mw=MF):
                for jj in range(JP):
                    nc.tensor.matmul(
                        ps[:],
                        w_tiles[nt][:, 2 * jj : 2 * jj + 2, :],
                        qx[:, 2 * jj : 2 * jj + 2, m0 : m0 + mw],
                        start=(jj == 0),
                        stop=(jj == JP - 1),
                        perf_mode=DR,
                    )

            # epilogue: t = ps*alpha + bias/(2os); q8 = fp8(clamp t);
            # y = q8 * 2os
            def emit_epilogue(ps, nt, mb, m0=0, mw=MF):
                t = epi.tile([P, mw], F32, tag="t", name="t")
                nc.scalar.activation(
                    t[:], ps[:], AF.Identity,
                    bias=cst[:, nt : nt + 1], scale=cst[:, NT : NT + 1],
                )
                q8 = q8p.tile([P, mw], FP8, tag="q8", name="q8")
                nc.vector.tensor_scalar(
                    q8[:], t[:], -224.0, 224.0, OP.max, OP.min
                )
                y = yp.tile([P, mw], F32, tag="y", name="y")
                nc.vector.tensor_scalar_mul(y[:], q8[:], cst[:, NT + 1 : NT + 2])
                oeng = nc.sync if (mw < MF and (m0 // mw) % 2 == 0) else nc.scalar
                oeng.dma_start(
                    out_t[nt * P : (nt + 1) * P, mb * MF + m0 : mb * MF + m0 + mw],
                    y[:],
                )

            # ---- PE clock pre-warm: the core idles at 1.2GHz and only
            # ramps to 2.4GHz after sustained tensor activity; burn tiny
            # matmuls on a zeroed tile while the first loads are in
            # flight so the real warm-up runs at full clock ----
            # Narrow ticks only: do NOT pre-ramp the clock to full — the
            # warm phase is DMA-delivery-bound, and a pre-ramped PE just
            # starves sooner and takes the idle-down penalty. The natural
            # half-clock start paces the PE to the delivery rate.
            dummy = consts.tile([P, 32], FP8)
            nc.vector.memset(dummy[:], 0.0)
            ps_dummy = psp.tile([32, 32], F32, tag="ps", name="psd")
            for _ in range(220):
                nc.tensor.matmul(
                    ps_dummy[:], dummy[:], dummy[:], start=True, stop=True
                )

            # ---- warm-up: k-outer across NW psum banks so the PE issues
            # NW matmuls per arriving qx quarter instead of idling ----
            ps_warm = [
                psp.tile([P, MF], F32, tag="ps", name=f"psw{g}")
                for g in range(NW)
            ]
            for jj in range(JP):
                for g in range(NW):
                    nc.tensor.matmul(
                        ps_warm[g][:],
                        w_tiles[g][:, 2 * jj : 2 * jj + 2, :],
                        qx_tiles[0][:, 2 * jj : 2 * jj + 2, :],
                        start=(jj == 0),
                        stop=(jj == JP - 1),
                        perf_mode=DR,
                    )
            for g in range(NW):
                emit_epilogue(ps_warm[g], g, 0)

            # ---- main loop over m blocks ----
            for mb in range(MB):
                qx = qx_tiles[mb]
                if mb + 1 < MB:
                    qx_tiles[mb + 1] = qxp.tile(
                        [P, KS, MF], FP8, tag="qx", name=f"qx{mb+1}"
                    )
                nt_range = list(range(NW, NT)) if mb == 0 else list(range(NT))
                nsteps = len(nt_range)
                for idx, nt in enumerate(nt_range):
                    last_tile = (mb == MB - 1) and (nt == NT - 1)
                    if last_tile:
                        # split the final tile so its epilogue+store tail
                        # overlaps the earlier slices' matmuls
                        for m0 in range(0, MF, MF // 4):
                            ps = psp.tile([P, MF // 4], F32, tag="ps", name="ps")
                            emit_mms(ps, nt, qx, m0, MF // 4)
                            emit_epilogue(ps, nt, mb, m0, MF // 4)
                    else:
                        ps = psp.tile([P, MF], F32, tag="ps", name="ps")
                        emit_mms(ps, nt, qx)
                        emit_epilogue(ps, nt, mb)
                    # prefetch next block's qx (one big 16KB-line descriptor)
                    if mb + 1 < MB and idx == nsteps // 2:
                        nc.sync.dma_start(
                            qx_tiles[mb + 1][:], qxt[mb + 1, :, :, :]
                        )
    return split_sync_waits(nc)


def prep_weight(weight):
    """[N, K] f32 (e4m3fn-grid values) -> [NT, 128, KS, 128] TRN-fp8 of w/2."""
    N, K = weight.shape
    wq = (weight.astype(np.float32) * 0.5).astype(NP_FP8)
    # [nt, n, j, p] -> [nt, p, j, n]
    return np.ascontiguousarray(
        wq.reshape(N // P, P, K // P, P).transpose(0, 3, 2, 1)
    )


def prep_consts(weight_scale, bias, input_scale, output_scale):
    """Packed [P, NT+2] per-partition constants (single contiguous DMA)."""
    si = float(np.asarray(input_scale, np.float64))
    sw = float(np.asarray(weight_scale, np.float64))
    os_ = float(np.asarray(output_scale, np.float64))
    N = bias.shape[0]
    NT = N // P
    bias2 = (bias.astype(np.float64) / (2.0 * os_)).astype(np.float32)
    cst = np.empty((P, NT + 2), np.float32)
    cst[:, 0:NT] = bias2.reshape(NT, P).T  # cst[p, nt] = bias2[nt*128+p]
    cst[:, NT] = np.float32(2.0 * si * sw / os_)
    cst[:, NT + 1] = np.float32(2.0 * os_)
    return np.ascontiguousarray(cst)


def kernel(x, weight, weight_scale, bias, input_scale, output_scale):
    x = np.asarray(x, np.float32)
    weight = np.asarray(weight, np.float32)
    bias = np.asarray(bias, np.float32)
    B, S, K = x.shape
    N = weight.shape[0]
    M_total = B * S
    M = M_total // N_CORES
    MF = 512
    MB = M // MF
    KS = K // P

    # Host-side static fp8 quantize of x, identical math to the reference:
    # clip(x/s_in, +-448) on the OCP grid == 2 * fp8_trn(clip(x/(2si), +-224)).
    # f32 multiply-by-reciprocal + RNE cast, matching the device DVE path.
    si = float(np.asarray(input_scale, np.float64))
    inv2si = np.float32(1.0 / (2.0 * si))
    xq = np.clip(x.reshape(M_total, K) * inv2si, -224.0, 224.0).astype(NP_FP8)
    # [c, mb, p, j, m] = xq[c*M + mb*MF + m, j*128 + p]
    qxt_all = np.ascontiguousarray(
        xq.reshape(N_CORES, MB, MF, KS, P).transpose(0, 1, 4, 3, 2)
    )

    wt = prep_weight(weight)
    cst = prep_consts(weight_scale, bias, input_scale, output_scale)

    nc = build(K, M, N, MF=MF)

    in_maps = []
    for c in range(N_CORES):
        in_maps.append({
            "qxt": qxt_all[c],
            "wt": wt,
            "cst": cst,
        })

    res = None
    last_exc = None
    for attempt in range(3):
        try:
            res = run_bass_kernel_spmd(nc, in_maps, core_ids=list(range(N_CORES)))
            break
        except Exception as e:  # transient NRT/device errors: retry
            last_exc = e
    if res is None:
        raise last_exc
    global LAST_RESULT
    LAST_RESULT = res

    out = np.empty((M_total, N), np.float32)
    for c in range(N_CORES):
        out[c * M : (c + 1) * M, :] = res.results[c]["out_t"].T
    return out.reshape(B, S, N)


# revision 26
# speedup vs baseline: 1.0005x; 1.0005x over previous
"""FP8StaticLinear Trainium2 kernel.

out = requant_fp8(qdq_fp8(x, s_in) @ (w * s_w).T + bias, s_out)

Sharding: data-parallel over tokens (B*S=16384 -> 2048/core on 8 cores).
Device math: fp8e4 DoubleRow matmuls on the PE array. Both operands are
halved on entry so the OCP-e4m3fn grid (max 448) maps onto TRN fp8e4
(max 240) exactly; scales are folded back in the epilogue.

x is quantized to fp8 on the host (exact same RNE rounding as the
device DVE path) and uploaded pre-tiled, which cuts the activation DMA
4x and removes the on-device quantize pipeline entirely; the warm-up
phase is then PE-bound instead of DMA-starved.
"""

import numpy as np
import ml_dtypes

# bass_utils imports antenv.axon_hooks lazily when tracing; some images
# lack it, which would crash the run under BASS_TRACE. Pre-inject a
# no-op stub so tracing degrades gracefully instead.
try:
    import antenv.axon_hooks  # noqa: F401
except Exception:
    import sys as _sys
    import types as _types

    _m = _types.ModuleType("antenv.axon_hooks")
    _m._HOOK = None
    _m.set_axon_ntff_profile_hook = lambda h: setattr(_m, "_HOOK", h)
    _m.get_axon_ntff_profile_hook = lambda: _m._HOOK
    _sys.modules["antenv.axon_hooks"] = _m
    try:
        import antenv as _antenv

        _antenv.axon_hooks = _m
    except Exception:
        pass

import concourse.bass as bass
import concourse.mybir as mybir
from concourse.tile import TileContext
from concourse.vector_clock import ScopedClock
from concourse.bass_utils import run_bass_kernel_spmd

FP8 = mybir.dt.float8e4
F32 = mybir.dt.float32
NP_FP8 = ml_dtypes.float8_e4m3  # TRN fp8e4 (max 240, has inf)

N_CORES = 8
P = 128


# ---------------------------------------------------------------------------
# Workaround: this walrus build rejects >1 sem-wait on the Tile tail Drain
# ("Too many sync wait commands"). Split the waits across single-wait drains.
def _drain_and_barrier(self, tick_clock, wait_clock):
    drain_inst = self.nc.sync.drain()
    wait_clock.add_sem_waits(
        drain_inst.ins, ScopedClock({None: tick_clock.global_clock})
    )
    w = list(drain_inst.ins.sync_info.on_wait)
    if len(w) > 1:
        drain_inst.ins.sync_info = mybir.SyncInfo(on_wait=[w[0]], on_update=[])
        for extra in w[1:]:
            d2 = self.nc.sync.drain()
            d2.ins.sync_info = mybir.SyncInfo(on_wait=[extra], on_update=[])
    self.nc.all_engine_barrier()
    assert self.sems is not None
    popped = self.nc._tile_sem_poison_stack.pop()
    assert popped is self._sem_poison
    self.nc.clear_and_free_semaphores(list(self.sems.allocated().values()))
    self.nc.all_engine_barrier()


TileContext._drain_and_barrier = _drain_and_barrier


def split_sync_waits(nc, max_waits=1):
    """Hoist extra sem-waits onto standalone EventSemaphore carriers."""
    n_new = 0
    for f in nc.m.functions:
        for blk in f.blocks:
            out = []
            changed = False
            for inst in blk.instructions:
                si = inst.sync_info
                w = list(si.on_wait) if si is not None else []
                if len(w) > max_waits:
                    upd = list(inst.sync_info.on_update)
                    for wi in w[max_waits:]:
                        es = mybir.InstEventSemaphore(
                            name=f"hoistw-{n_new}", ins=[], outs=[]
                        )
                        n_new += 1
                        es.engine = inst.engine
                        es.sync_info = mybir.SyncInfo(on_wait=[wi], on_update=[])
                        out.append(es)
                    inst.sync_info = mybir.SyncInfo(
                        on_wait=w[:max_waits], on_update=upd
                    )
                    changed = True
                out.append(inst)
            if changed:
                blk.instructions = out
    return nc
# ---------------------------------------------------------------------------

DR = mybir.MatmulPerfMode.DoubleRow


def build(K, M, N, MF=512):
    """One-core program: out_t[N, M] = requantized (x @ w.T + b) transposed.

    DRAM inputs:
      qxt    [MB, P, KS, MF] fp8   halved-quantized x, block-tiled:
                     qxt[mb, p, j, m] = fp8(clip(x[mb*MF+m, j*128+p]/(2si)))
      wt     [NT, P, KS, P]  fp8   halved weight, tiled:
                     wt[nt, p, j, n] = fp8(w[nt*128+n, j*128+p] / 2)
      bias2  [N]     f32    bias / (2*s_out)
      alpha, two_os  [1, 1] f32:   2*s_in*s_w/s_out,  2*s_out
    Output:
      out_t  [N, M]  f32
    """
    KS = K // P          # 32 k subtiles of 128
    JP = KS // 2         # 16 DoubleRow pairs
    NT = N // P          # 32 n tiles
    MB = M // MF         # 4 m blocks
    NW = min(8, NT)      # warm groups (one psum bank each)
    QW = KS // 4         # k-subtiles per quarter piece
    AF = mybir.ActivationFunctionType
    OP = mybir.AluOpType

    nc = bass.Bass()
    qxt = nc.dram_tensor("qxt", [MB, P, KS, MF], FP8, kind="ExternalInput")
    wt = nc.dram_tensor("wt", [NT, P, KS, P], FP8, kind="ExternalInput")
    # packed per-partition constants: [:, 0:NT] = bias/(2os) tiled,
    # [:, NT] = alpha, [:, NT+1] = 2os
    cst_d = nc.dram_tensor("cst", [P, NT + 2], F32, kind="ExternalInput")
    out_t = nc.dram_tensor("out_t", [N, M], F32, kind="ExternalOutput")

    with TileContext(nc) as tc:
        with (
            tc.tile_pool(name="consts", bufs=1) as consts,
            tc.tile_pool(name="wres", bufs=1) as wres,
            tc.tile_pool(name="qx", bufs=2) as qxp,
            tc.tile_pool(name="psum", bufs=8, space="PSUM") as psp,
            tc.tile_pool(name="epi", bufs=3) as epi,
            tc.tile_pool(name="q8", bufs=3) as q8p,
            tc.tile_pool(name="yout", bufs=4) as yp,
        ):
            # ---- consts: one contiguous DMA on the gpsimd SWDGE queue ----
            cst = consts.tile([P, NT + 2], F32)
            nc.gpsimd.dma_start(cst[:], cst_d[:, :])

            w_tiles = [
                wres.tile([P, KS, P], FP8, tag=f"w{nt}", name=f"w{nt}")
                for nt in range(NT)
            ]
            qx_tiles = {0: qxp.tile([P, KS, MF], FP8, tag="qx", name="qx0")}

            # Loads: qx on the sync HW queue, warm weights on the scalar
            # HW queue (few issues, done long before the first epilogue
            # ACTIVATE needs the engine — a long issue backlog there
            # blocks PSUM recycling). First pieces are small so the first
            # matmul fires as early as possible.
            HW = KS // 2
            for h in range(2):
                nc.sync.dma_start(
                    qx_tiles[0][:, 2 * h * QW : (2 * h + 1) * QW, :],
                    qxt[0, :, 2 * h * QW : (2 * h + 1) * QW, :],
                )
                nc.sync.dma_start(
                    qx_tiles[0][:, (2 * h + 1) * QW : (2 * h + 2) * QW, :],
                    qxt[0, :, (2 * h + 1) * QW : (2 * h + 2) * QW, :],
                )
                for g in range(NW):
                    nc.sync.dma_start(
                        w_tiles[g][:, h * HW : (h + 1) * HW, :],
                        wt[g, :, h * HW : (h + 1) * HW, :],
                    )
            # Remaining weight tiles: one big descriptor each (4KB lines).
            for nt in range(NW, NT):
                nc.sync.dma_start(w_tiles[nt][:], wt[nt, :, :, :])

            def emit_mms(ps, nt, qx, m0=0, mw=MF):
                for jj in range(JP):
                    nc.tensor.matmul(
                        ps[:],
                        w_tiles[nt][:, 2 * jj : 2 * jj + 2, :],
                        qx[:, 2 * jj : 2 * jj + 2, m0 : m0 + mw],
                        start=(jj == 0),
                        stop=(jj == JP - 1),
                        perf_mode=DR,
                    )

            # epilogue: t = ps*alpha + bias/(2os); q8 = fp8(clamp t);
            # y = q8 * 2os
            def emit_epilogue(ps, nt, mb, m0=0, mw=MF):
                t = epi.tile([P, mw], F32, tag="t", name="t")
                nc.scalar.activation(
                    t[:], ps[:], AF.Identity,
                    bias=cst[:, nt : nt + 1], scale=cst[:, NT : NT + 1],
                )
                q8 = q8p.tile([P, mw], FP8, tag="q8", name="q8")
                nc.vector.tensor_scalar(
                    q8[:], t[:], -224.0, 224.0, OP.max, OP.min
                )
                y = yp.tile([P, mw], F32, tag="y", name="y")
                nc.vector.tensor_scalar_mul(y[:], q8[:], cst[:, NT + 1 : NT + 2])
                oeng = nc.sync if (mw < MF and (m0 // mw) % 2 == 0) else nc.scalar
                oeng.dma_start(
                    out_t[nt * P : (nt + 1) * P, mb * MF + m0 : mb * MF + m0 + mw],
                    y[:],
                )

            # ---- PE clock pre-warm: the core idles at 1.2GHz and only
            # ramps to 2.4GHz after sustained tensor activity; burn tiny
            # matmuls on a zeroed tile while the first loads are in
            # flight so the real warm-up runs at full clock ----
            # Narrow ticks only: do NOT pre-ramp the clock to full — the
            # warm phase is DMA-delivery-bound, and a pre-ramped PE just
            # starves sooner and takes the idle-down penalty. The natural
            # half-clock start paces the PE to the delivery rate.
            dummy = consts.tile([P, 32], FP8)
            nc.vector.memset(dummy[:], 0.0)
            ps_dummy = psp.tile([32, 32], F32, tag="ps", name="psd")
            for _ in range(160):
                nc.tensor.matmul(
                    ps_dummy[:], dummy[:], dummy[:], start=True, stop=True
                )

            # ---- warm-up: k-outer across NW psum banks so the PE issues
            # NW matmuls per arriving qx quarter instead of idling ----
            ps_warm = [
                psp.tile([P, MF], F32, tag="ps", name=f"psw{g}")
                for g in range(NW)
            ]
            for jj in range(JP):
                for g in range(NW):
                    nc.tensor.matmul(
                        ps_warm[g][:],
                        w_tiles[g][:, 2 * jj : 2 * jj + 2, :],
                        qx_tiles[0][:, 2 * jj : 2 * jj + 2, :],
                        start=(jj == 0),
                        stop=(jj == JP - 1),
                        perf_mode=DR,
                    )
            for g in range(NW):
                emit_epilogue(ps_warm[g], g, 0)

            # ---- main loop over m blocks ----
            for mb in range(MB):
                qx = qx_tiles[mb]
                if mb + 1 < MB:
                    qx_tiles[mb + 1] = qxp.tile(
                        [P, KS, MF], FP8, tag="qx", name=f"qx{mb+1}"
                    )
                nt_range = list(range(NW, NT)) if mb == 0 else list(range(NT))
                nsteps = len(nt_range)
                for idx, nt in enumerate(nt_range):
                    last_tile = (mb == MB - 1) and (nt == NT - 1)
                    if last_tile:
                        # split the final tile so its epilogue+store tail
                        # overlaps the earlier slices' matmuls
                        for m0 in range(0, MF, MF // 4):
                            ps = psp.tile([P, MF // 4], F32, tag="ps", name="ps")
                            emit_mms(ps, nt, qx, m0, MF // 4)
                            emit_epilogue(ps, nt, mb, m0, MF // 4)
                    else:
                        ps = psp.tile([P, MF], F32, tag="ps", name="ps")
                        emit_mms(ps, nt, qx)
                        emit_epilogue(ps, nt, mb)
                    # prefetch next block's qx (one big 16KB-line descriptor)
                    if mb + 1 < MB and idx == nsteps // 2:
                        nc.sync.dma_start(
                            qx_tiles[mb + 1][:], qxt[mb + 1, :, :, :]
                        )
    return split_sync_waits(nc)


def prep_weight(weight):
    """[N, K] f32 (e4m3fn-grid values) -> [NT, 128, KS, 128] TRN-fp8 of w/2."""
    N, K = weight.shape
    wq = (weight.astype(np.float32) * 0.5).astype(NP_FP8)
    # [nt, n, j, p] -> [nt, p, j, n]
    return np.ascontiguousarray(
        wq.reshape(N // P, P, K // P, P).transpose(0, 3, 2, 1)
    )


def prep_consts(weight_scale, bias, input_scale, output_scale):
    """Packed [P, NT+2] per-partition constants (single contiguous DMA)."""
    si = float(np.asarray(input_scale, np.float64))
    sw = float(np.asarray(weight_scale, np.float64))
    os_ = float(np.asarray(output_scale, np.float64))
    N = bias.shape[0]
    NT = N // P
    bias2 = (bias.astype(np.float64) / (2.0 * os_)).astype(np.float32)
    cst = np.empty((P, NT + 2), np.float32)
    cst[:, 0:NT] = bias2.reshape(NT, P).T  # cst[p, nt] = bias2[nt*128+p]
    cst[:, NT] = np.float32(2.0 * si * sw / os_)
    cst[:, NT + 1] = np.float32(2.0 * os_)
    return np.ascontiguousarray(cst)


def kernel(x, weight, weight_scale, bias, input_scale, output_scale):
    x = np.asarray(x, np.float32)
    weight = np.asarray(weight, np.float32)
    bias = np.asarray(bias, np.float32)
    B, S, K = x.shape
    N = weight.shape[0]
    M_total = B * S
    M = M_total // N_CORES
    MF = 512
    MB = M // MF
    KS = K // P

    # Host-side static fp8 quantize of x, identical math to the reference:
    # clip(x/s_in, +-448) on the OCP grid == 2 * fp8_trn(clip(x/(2si), +-224)).
    # f32 multiply-by-reciprocal + RNE cast, matching the device DVE path.
    si = float(np.asarray(input_scale, np.float64))
    inv2si = np.float32(1.0 / (2.0 * si))
    xq = np.clip(x.reshape(M_total, K) * inv2si, -224.0, 224.0).astype(NP_FP8)
    # [c, mb, p, j, m] = xq[c*M + mb*MF + m, j*128 + p]
    qxt_all = np.ascontiguousarray(
        xq.reshape(N_CORES, MB, MF, KS, P).transpose(0, 1, 4, 3, 2)
    )

    wt = prep_weight(weight)
    cst = prep_consts(weight_scale, bias, input_scale, output_scale)

    nc = build(K, M, N, MF=MF)

    in_maps = []
    for c in range(N_CORES):
        in_maps.append({
            "qxt": qxt_all[c],
            "wt": wt,
            "cst": cst,
        })

    res = None
    last_exc = None
    for attempt in range(3):
        try:
            res = run_bass_kernel_spmd(nc, in_maps, core_ids=list(range(N_CORES)))
            break
        except Exception as e:  # transient NRT/device errors: retry
            last_exc = e
    if res is None:
        raise last_exc
    global LAST_RESULT
    LAST_RESULT = res

    out = np.empty((M_total, N), np.float32)
    for c in range(N_CORES):
        out[c * M : (c + 1) * M, :] = res.results[c]["out_t"].T
    return out.reshape(B, S, N)


# revision 27
# speedup vs baseline: 1.0055x; 1.0050x over previous
"""FP8StaticLinear Trainium2 kernel.

out = requant_fp8(qdq_fp8(x, s_in) @ (w * s_w).T + bias, s_out)

Sharding: data-parallel over tokens (B*S=16384 -> 2048/core on 8 cores).
Device math: fp8e4 DoubleRow matmuls on the PE array. Both operands are
halved on entry so the OCP-e4m3fn grid (max 448) maps onto TRN fp8e4
(max 240) exactly; scales are folded back in the epilogue.

x is quantized to fp8 on the host (exact same RNE rounding as the
device DVE path) and uploaded pre-tiled, which cuts the activation DMA
4x and removes the on-device quantize pipeline entirely; the warm-up
phase is then PE-bound instead of DMA-starved.
"""

import numpy as np
import ml_dtypes

# bass_utils imports antenv.axon_hooks lazily when tracing; some images
# lack it, which would crash the run under BASS_TRACE. Pre-inject a
# no-op stub so tracing degrades gracefully instead.
try:
    import antenv.axon_hooks  # noqa: F401
except Exception:
    import sys as _sys
    import types as _types

    _m = _types.ModuleType("antenv.axon_hooks")
    _m._HOOK = None
    _m.set_axon_ntff_profile_hook = lambda h: setattr(_m, "_HOOK", h)
    _m.get_axon_ntff_profile_hook = lambda: _m._HOOK
    _sys.modules["antenv.axon_hooks"] = _m
    try:
        import antenv as _antenv

        _antenv.axon_hooks = _m
    except Exception:
        pass

import concourse.bass as bass
import concourse.mybir as mybir
from concourse.tile import TileContext
from concourse.vector_clock import ScopedClock
from concourse.bass_utils import run_bass_kernel_spmd

FP8 = mybir.dt.float8e4
F32 = mybir.dt.float32
NP_FP8 = ml_dtypes.float8_e4m3  # TRN fp8e4 (max 240, has inf)

N_CORES = 8
P = 128


# ---------------------------------------------------------------------------
# Workaround: this walrus build rejects >1 sem-wait on the Tile tail Drain
# ("Too many sync wait commands"). Split the waits across single-wait drains.
def _drain_and_barrier(self, tick_clock, wait_clock):
    drain_inst = self.nc.sync.drain()
    wait_clock.add_sem_waits(
        drain_inst.ins, ScopedClock({None: tick_clock.global_clock})
    )
    w = list(drain_inst.ins.sync_info.on_wait)
    if len(w) > 1:
        drain_inst.ins.sync_info = mybir.SyncInfo(on_wait=[w[0]], on_update=[])
        for extra in w[1:]:
            d2 = self.nc.sync.drain()
            d2.ins.sync_info = mybir.SyncInfo(on_wait=[extra], on_update=[])
    self.nc.all_engine_barrier()
    assert self.sems is not None
    popped = self.nc._tile_sem_poison_stack.pop()
    assert popped is self._sem_poison
    self.nc.clear_and_free_semaphores(list(self.sems.allocated().values()))
    self.nc.all_engine_barrier()


TileContext._drain_and_barrier = _drain_and_barrier


def split_sync_waits(nc, max_waits=1):
    """Hoist extra sem-waits onto standalone EventSemaphore carriers."""
    n_new = 0
    for f in nc.m.functions:
        for blk in f.blocks:
            out = []
            changed = False
            for inst in blk.instructions:
                si = inst.sync_info
                w = list(si.on_wait) if si is not None else []
                if len(w) > max_waits:
                    upd = list(inst.sync_info.on_update)
                    for wi in w[max_waits:]:
                        es = mybir.InstEventSemaphore(
                            name=f"hoistw-{n_new}", ins=[], outs=[]
                        )
                        n_new += 1
                        es.engine = inst.engine
                        es.sync_info = mybir.SyncInfo(on_wait=[wi], on_update=[])
                        out.append(es)
                    inst.sync_info = mybir.SyncInfo(
                        on_wait=w[:max_waits], on_update=upd
                    )
                    changed = True
                out.append(inst)
            if changed:
                blk.instructions = out
    return nc
# ---------------------------------------------------------------------------

DR = mybir.MatmulPerfMode.DoubleRow


def build(K, M, N, MF=512):
    """One-core program: out_t[N, M] = requantized (x @ w.T + b) transposed.

    DRAM inputs:
      qxt    [MB, P, KS, MF] fp8   halved-quantized x, block-tiled:
                     qxt[mb, p, j, m] = fp8(clip(x[mb*MF+m, j*128+p]/(2si)))
      wt     [NT, P, KS, P]  fp8   halved weight, tiled:
                     wt[nt, p, j, n] = fp8(w[nt*128+n, j*128+p] / 2)
      bias2  [N]     f32    bias / (2*s_out)
      alpha, two_os  [1, 1] f32:   2*s_in*s_w/s_out,  2*s_out
    Output:
      out_t  [N, M]  f32
    """
    KS = K // P          # 32 k subtiles of 128
    JP = KS // 2         # 16 DoubleRow pairs
    NT = N // P          # 32 n tiles
    MB = M // MF         # 4 m blocks
    NW = min(8, NT)      # warm groups (one psum bank each)
    QW = KS // 4         # k-subtiles per quarter piece
    AF = mybir.ActivationFunctionType
    OP = mybir.AluOpType

    nc = bass.Bass()
    qxt = nc.dram_tensor("qxt", [MB, P, KS, MF], FP8, kind="ExternalInput")
    wt = nc.dram_tensor("wt", [NT, P, KS, P], FP8, kind="ExternalInput")
    # packed per-partition constants: [:, 0:NT] = bias/(2os) tiled,
    # [:, NT] = alpha, [:, NT+1] = 2os
    cst_d = nc.dram_tensor("cst", [P, NT + 2], F32, kind="ExternalInput")
    out_t = nc.dram_tensor("out_t", [N, M], F32, kind="ExternalOutput")

    with TileContext(nc) as tc:
        with (
            tc.tile_pool(name="consts", bufs=1) as consts,
            tc.tile_pool(name="wres", bufs=1) as wres,
            tc.tile_pool(name="qx", bufs=2) as qxp,
            tc.tile_pool(name="psum", bufs=8, space="PSUM") as psp,
            tc.tile_pool(name="epi", bufs=3) as epi,
            tc.tile_pool(name="q8", bufs=3) as q8p,
            tc.tile_pool(name="yout", bufs=4) as yp,
        ):
            # ---- consts: one contiguous DMA on the gpsimd SWDGE queue ----
            cst = consts.tile([P, NT + 2], F32)
            nc.gpsimd.dma_start(cst[:], cst_d[:, :])

            w_tiles = [
                wres.tile([P, KS, P], FP8, tag=f"w{nt}", name=f"w{nt}")
                for nt in range(NT)
            ]
            qx_tiles = {0: qxp.tile([P, KS, MF], FP8, tag="qx", name="qx0")}

            # Loads: qx on the sync HW queue, warm weights on the scalar
            # HW queue (few issues, done long before the first epilogue
            # ACTIVATE needs the engine — a long issue backlog there
            # blocks PSUM recycling). First pieces are small so the first
            # matmul fires as early as possible.
            HW = KS // 2
            for h in range(2):
                nc.sync.dma_start(
                    qx_tiles[0][:, 2 * h * QW : (2 * h + 1) * QW, :],
                    qxt[0, :, 2 * h * QW : (2 * h + 1) * QW, :],
                )
                nc.sync.dma_start(
                    qx_tiles[0][:, (2 * h + 1) * QW : (2 * h + 2) * QW, :],
                    qxt[0, :, (2 * h + 1) * QW : (2 * h + 2) * QW, :],
                )
                for g in range(NW):
                    nc.sync.dma_start(
                        w_tiles[g][:, h * HW : (h + 1) * HW, :],
                        wt[g, :, h * HW : (h + 1) * HW, :],
                    )
            # Remaining weight tiles: one big descriptor each (4KB lines).
            for nt in range(NW, NT):
                nc.sync.dma_start(w_tiles[nt][:], wt[nt, :, :, :])

            def emit_mms(ps, nt, qx, m0=0, mw=MF):
                for jj in range(JP):
                    nc.tensor.matmul(
                        ps[:],
                        w_tiles[nt][:, 2 * jj : 2 * jj + 2, :],
                        qx[:, 2 * jj : 2 * jj + 2, m0 : m0 + mw],
                        start=(jj == 0),
                        stop=(jj == JP - 1),
                        perf_mode=DR,
                    )

            # epilogue: t = ps*alpha + bias/(2os); q8 = fp8(clamp t);
            # y = q8 * 2os
            def emit_epilogue(ps, nt, mb, m0=0, mw=MF):
                t = epi.tile([P, mw], F32, tag="t", name="t")
                nc.scalar.activation(
                    t[:], ps[:], AF.Identity,
                    bias=cst[:, nt : nt + 1], scale=cst[:, NT : NT + 1],
                )
                q8 = q8p.tile([P, mw], FP8, tag="q8", name="q8")
                nc.vector.tensor_scalar(
                    q8[:], t[:], -224.0, 224.0, OP.max, OP.min
                )
                y = yp.tile([P, mw], F32, tag="y", name="y")
                nc.vector.tensor_scalar_mul(y[:], q8[:], cst[:, NT + 1 : NT + 2])
                oeng = nc.sync if (mw < MF and (m0 // mw) % 2 == 0) else nc.scalar
                oeng.dma_start(
                    out_t[nt * P : (nt + 1) * P, mb * MF + m0 : mb * MF + m0 + mw],
                    y[:],
                )

            # ---- PE clock pre-warm: the core idles at 1.2GHz and only
            # ramps to 2.4GHz after sustained tensor activity; burn tiny
            # matmuls on a zeroed tile while the first loads are in
            # flight so the real warm-up runs at full clock ----
            # Narrow ticks only: do NOT pre-ramp the clock to full — the
            # warm phase is DMA-delivery-bound, and a pre-ramped PE just
            # starves sooner and takes the idle-down penalty. The natural
            # half-clock start paces the PE to the delivery rate.
            dummy = consts.tile([P, 32], FP8)
            nc.vector.memset(dummy[:], 0.0)
            ps_dummy = psp.tile([32, 32], F32, tag="ps", name="psd")
            for _ in range(160):
                nc.tensor.matmul(
                    ps_dummy[:], dummy[:], dummy[:], start=True, stop=True
                )

            # ---- warm-up: k-outer across NW psum banks so the PE issues
            # NW matmuls per arriving qx quarter instead of idling ----
            ps_warm = [
                psp.tile([P, MF], F32, tag="ps", name=f"psw{g}")
                for g in range(NW)
            ]
            for jj in range(JP):
                for g in range(NW):
                    nc.tensor.matmul(
                        ps_warm[g][:],
                        w_tiles[g][:, 2 * jj : 2 * jj + 2, :],
                        qx_tiles[0][:, 2 * jj : 2 * jj + 2, :],
                        start=(jj == 0),
                        stop=(jj == JP - 1),
                        perf_mode=DR,
                    )
            for g in range(NW):
                emit_epilogue(ps_warm[g], g, 0)

            # ---- main loop over m blocks ----
            for mb in range(MB):
                qx = qx_tiles[mb]
                if mb + 1 < MB:
                    qx_tiles[mb + 1] = qxp.tile(
                        [P, KS, MF], FP8, tag="qx", name=f"qx{mb+1}"
                    )
                nt_range = list(range(NW, NT)) if mb == 0 else list(range(NT))
                nsteps = len(nt_range)
                for idx, nt in enumerate(nt_range):
                    last_tile = (mb == MB - 1) and (nt == NT - 1)
                    if last_tile:
                        # split the final tile so its epilogue+store tail
                        # overlaps the second half's matmuls (FD stays at
                        # 256 where DoubleRow is still streaming-bound)
                        for m0 in range(0, MF, MF // 2):
                            ps = psp.tile([P, MF // 2], F32, tag="ps", name="ps")
                            emit_mms(ps, nt, qx, m0, MF // 2)
                            emit_epilogue(ps, nt, mb, m0, MF // 2)
                    else:
                        ps = psp.tile([P, MF], F32, tag="ps", name="ps")
                        emit_mms(ps, nt, qx)
                        emit_epilogue(ps, nt, mb)
                    # prefetch next block's qx (one big 16KB-line descriptor)
                    if mb + 1 < MB and idx == nsteps // 2:
                        nc.sync.dma_start(
                            qx_tiles[mb + 1][:], qxt[mb + 1, :, :, :]
                        )
    return split_sync_waits(nc)


def prep_weight(weight):
    """[N, K] f32 (e4m3fn-grid values) -> [NT, 128, KS, 128] TRN-fp8 of w/2."""
    N, K = weight.shape
    wq = (weight.astype(np.float32) * 0.5).astype(NP_FP8)
    # [nt, n, j, p] -> [nt, p, j, n]
    return np.ascontiguousarray(
        wq.reshape(N // P, P, K // P, P).transpose(0, 3, 2, 1)
    )


def prep_consts(weight_scale, bias, input_scale, output_scale):
    """Packed [P, NT+2] per-partition constants (single contiguous DMA)."""
    si = float(np.asarray(input_scale, np.float64))
    sw = float(np.asarray(weight_scale, np.float64))
    os_ = float(np.asarray(output_scale, np.float64))
    N = bias.shape[0]
    NT = N // P
    bias2 = (bias.astype(np.float64) / (2.0 * os_)).astype(np.float32)
    cst = np.empty((P, NT + 2), np.float32)
    cst[:, 0:NT] = bias2.reshape(NT, P).T  # cst[p, nt] = bias2[nt*128+p]
    cst[:, NT] = np.float32(2.0 * si * sw / os_)
    cst[:, NT + 1] = np.float32(2.0 * os_)
    return np.ascontiguousarray(cst)


def kernel(x, weight, weight_scale, bias, input_scale, output_scale):
    x = np.asarray(x, np.float32)
    weight = np.asarray(weight, np.float32)
    bias = np.asarray(bias, np.float32)
    B, S, K = x.shape
    N = weight.shape[0]
    M_total = B * S
    M = M_total // N_CORES
    MF = 512
    MB = M // MF
    KS = K // P

    # Host-side static fp8 quantize of x, identical math to the reference:
    # clip(x/s_in, +-448) on the OCP grid == 2 * fp8_trn(clip(x/(2si), +-224)).
    # f32 multiply-by-reciprocal + RNE cast, matching the device DVE path.
    si = float(np.asarray(input_scale, np.float64))
    inv2si = np.float32(1.0 / (2.0 * si))
    xq = np.clip(x.reshape(M_total, K) * inv2si, -224.0, 224.0).astype(NP_FP8)
    # [c, mb, p, j, m] = xq[c*M + mb*MF + m, j*128 + p]
    qxt_all = np.ascontiguousarray(
        xq.reshape(N_CORES, MB, MF, KS, P).transpose(0, 1, 4, 3, 2)
    )

    wt = prep_weight(weight)
    cst = prep_consts(weight_scale, bias, input_scale, output_scale)

    nc = build(K, M, N, MF=MF)

    in_maps = []
    for c in range(N_CORES):
        in_maps.append({
            "qxt": qxt_all[c],
            "wt": wt,
            "cst": cst,
        })

    res = None
    last_exc = None
    for attempt in range(3):
        try:
            res = run_bass_kernel_spmd(nc, in_maps, core_ids=list(range(N_CORES)))
            break
        except Exception as e:  # transient NRT/device errors: retry
            last_exc = e
    if res is None:
        raise last_exc
    global LAST_RESULT
    LAST_RESULT = res

    out = np.empty((M_total, N), np.float32)
    for c in range(N_CORES):
        out[c * M : (c + 1) * M, :] = res.results[c]["out_t"].T
    return out.reshape(B, S, N)
